# revision 31
# baseline (speedup 1.0000x reference)
"""Trainium2 Bass kernel for MoE-routed embedding MLP (nn_KML_24300924961295).

Model (B=4096, E=64 experts, D=H=256, vocab 100000):
    x = emb_table[entity_ids]                    # [B, D]
    h = tanh(x @ W1[rel] + b1[rel])              # [B, H]
    y = h @ W2[rel] + b2[rel]                    # [B, D]
    out = y / ||y||_2 (row-wise)

Sharding: experts are sharded across the 8 cores (core c owns experts
8c..8c+7); samples are routed on the host to the core owning their
relation, each expert group padded to a fixed capacity of C=128 samples
so all cores run one identical SPMD program.  The embedding rows are
gathered AND transposed on the host (X^T per expert), so the device
sees dense bf16 operands and does no indirect DMA and no PE transposes.

Per-core device pipeline (all matmul operands bf16, PSUM fp32), for
each pair of experts (2j, 2j+1):
    H^T [h,c] <- matmul(lhsT=W1 chunk, rhs=X^T chunk) accum over d,
                 + rank-1 bias matmul (b1 row x ones)      -> ps_h2
    ht        <- one ACT Tanh over the whole [128, 512] pair tile
    Y   [c,d] <- matmul(lhsT=H^T chunk, rhs=W2 rows) + rank-1 (ones x b2)
    s2  [c,1] <- DVE tensor_tensor_reduce(psy * psy)  (row sum of squares)
Then per half (4 experts): rsqrt on DVE only (0x5f3759df magic seed +
2 Newton steps), per-expert scale on ACT (Copy w/ per-partition scale,
fp32 PSUM -> bf16 SBUF), one 256 KiB output DMA.  Host upcasts to fp32.
"""

import numpy as np
from contextlib import ExitStack

import ml_dtypes

# ---- problem constants (hardcoded per the task contract) ----
B = 4096
E = 64
D = 256
HD = 256
N_CORES = 8
NE = E // N_CORES          # experts per core
C = 128                    # capacity (samples) per expert
HALF = NE // 2

BF16 = ml_dtypes.bfloat16
RSQRT_MAGIC = 0x5F3759DF

_compiled = {}


def _build_nc():
    """Build + schedule the single-core SPMD Bass program."""
    import concourse.bass as bass  # noqa: F401  (kept for parity with docs)
    import concourse.bacc as bacc
    import concourse.tile as tile
    from concourse import mybir

    fp32 = mybir.dt.float32
    bf16 = mybir.dt.bfloat16
    u32 = mybir.dt.uint32
    AF = mybir.ActivationFunctionType
    ALU = mybir.AluOpType

    nc = bacc.Bacc("TRN2", target_bir_lowering=False, debug=False)

    NEP = NE // 2  # expert pairs per core

    # X^T: [d-in-chunk(128 part), expert, d-chunk, sample]
    xt_in = nc.dram_tensor("xt", [128, NE, 2, C], bf16, kind="ExternalInput").ap()
    # partition-major weights: [p, e, 0:2 W1 K-chunks | 2:4 W2 H-chunks, :]
    w12 = nc.dram_tensor("w12", [128, NE, 4, HD], bf16, kind="ExternalInput").ap()
    # packed biases: [:, e, 0:256] = b1 rows (2 h-chunks), [:, e, 256:512] = b2
    bias = nc.dram_tensor("bias", [1, NE, 2 * 128 + D], bf16, kind="ExternalInput").ap()
    # output row-major per sample slot: [sample, expert, D]
    y = nc.dram_tensor("y", [C, NE, D], bf16, kind="ExternalOutput").ap()

    with tile.TileContext(nc) as tc:
        with ExitStack() as ctx:
            const_pool = ctx.enter_context(tc.tile_pool(name="const", bufs=1))
            ht_pool = ctx.enter_context(tc.tile_pool(name="htp", bufs=3))
            psh_pool = ctx.enter_context(
                tc.tile_pool(name="psh", bufs=3, space="PSUM")
            )
            psy_pool = ctx.enter_context(
                tc.tile_pool(name="psy", bufs=1, space="PSUM")
            )
            sq_pool = ctx.enter_context(tc.tile_pool(name="sqp", bufs=2))

            # Two HWDGE rings in parallel.  The scalar (ACT) ring q10
            # sustains ~2x the bandwidth of the sync ring q1 on this part,
            # so the critical pair-0/1 inputs go there; bulk + stores on
            # sync.  Issue cost ~750ns per dma_start regardless of size.
            xt_all = const_pool.tile([128, NE, 2, C], bf16)
            w_all = const_pool.tile([128, NE, 4, HD], bf16)
            bias_sb = const_pool.tile([1, NE, 2 * 128 + D], bf16)

            # scalar ring: tiny consts, pair-0 X^T, weight pairs 0-1.
            nc.scalar.dma_start(bias_sb[:], bias[:])
            nc.scalar.dma_start(xt_all[:, 0:2], xt_in[:, 0:2])
            nc.scalar.dma_start(w_all[:, 0:2], w12[:, 0:2])
            nc.scalar.dma_start(w_all[:, 2:4], w12[:, 2:4])

            # sync ring: rest of X^T, weight pairs 2-3 (and stores later).
            nc.sync.dma_start(xt_all[:, 2:], xt_in[:, 2:])
            nc.sync.dma_start(w_all[:, 4:6], w12[:, 4:6])
            nc.sync.dma_start(w_all[:, 6:8], w12[:, 6:8])

            w_tiles = [w_all[:, j] for j in range(NE)]

            ones1 = const_pool.tile([1, C], bf16)
            nc.gpsimd.memset(ones1[:], 1.0)
            kmag = const_pool.tile([C, HALF], u32)
            nc.gpsimd.memset(kmag[:], RSQRT_MAGIC)

            s2_all = const_pool.tile([C, NE], fp32)
            out_sb = const_pool.tile([C, NE, D], bf16)

            psy_tiles = []

            def pair_body(t):
                """Experts 2t, 2t+1: H^T + tanh + Y + row sum-of-squares."""
                ps_h2 = psh_pool.tile([128, 2, 2, C], fp32, tag="psh2")
                for j2 in range(2):
                    j = 2 * t + j2
                    wt = w_tiles[j]
                    for hc in range(2):
                        for dc in range(2):
                            nc.tensor.matmul(
                                ps_h2[:, j2, hc, :],
                                lhsT=wt[:, dc, hc * 128 : (hc + 1) * 128],
                                rhs=xt_all[:, j, dc, :],
                                start=(dc == 0),
                                stop=False,
                            )
                        nc.tensor.matmul(
                            ps_h2[:, j2, hc, :],
                            lhsT=bias_sb[:, j, hc * 128 : (hc + 1) * 128],
                            rhs=ones1[:],
                            start=False,
                            stop=True,
                        )
                ht2 = ht_pool.tile([128, 2, 2, C], bf16)
                nc.scalar.activation(ht2[:], ps_h2[:], AF.Tanh)
                ps_y2 = psy_pool.tile([C, 2, D], fp32, tag=f"psy{t}")
                for j2 in range(2):
                    j = 2 * t + j2
                    wt = w_tiles[j]
                    ps_y = ps_y2[:, j2, :]
                    nc.tensor.matmul(
                        ps_y, lhsT=ht2[:, j2, 0, :], rhs=wt[:, 2, :],
                        start=True, stop=False,
                    )
                    nc.tensor.matmul(
                        ps_y, lhsT=ht2[:, j2, 1, :], rhs=wt[:, 3, :],
                        start=False, stop=False,
                    )
                    nc.tensor.matmul(
                        ps_y, lhsT=ones1[:], rhs=bias_sb[:, j, 256:],
                        start=False, stop=True,
                    )
                    psy_tiles.append(ps_y)
                    # ACT Square (+row accumulate); square is in the same
                    # table set as tanh, so no ACT table switch, and ACT
                    # has slack under the PE phase
                    sq = sq_pool.tile([C, D], bf16, tag="sqa")
                    nc.scalar.activation(
                        sq[:], ps_y, AF.Square,
                        accum_out=s2_all[:, j : j + 1],
                    )

            def norm_half(h):
                """rsqrt of s2 (DVE-only), ACT-scale the 4 experts, store."""
                sl = slice(h * HALF, (h + 1) * HALF)
                s2u = s2_all[:, sl].bitcast(u32)
                sh = const_pool.tile([C, HALF], u32, tag=f"sh{h}")
                nc.vector.tensor_scalar(
                    out=sh[:], in0=s2u, scalar1=1, scalar2=None,
                    op0=ALU.logical_shift_right,
                )
                sd = const_pool.tile([C, HALF], u32, tag=f"sd{h}")
                nc.vector.tensor_tensor(
                    out=sd[:], in0=kmag[:], in1=sh[:], op=ALU.subtract
                )
                cur = sd[:].bitcast(fp32)
                s2 = s2_all[:, sl]
                # one Newton step: r' = r*(1.5 - 0.5*s2*r^2) -> ~0.2% max
                # rel err on the row norm, well inside the error budget
                for it in range(1):
                    u = const_pool.tile([C, HALF], fp32, tag=f"nt{h}{it}u")
                    nc.vector.tensor_mul(u[:], cur, s2)
                    v = const_pool.tile([C, HALF], fp32, tag=f"nt{h}{it}v")
                    nc.vector.scalar_tensor_tensor(
                        out=v[:], in0=u[:], scalar=-0.5, in1=cur,
                        op0=ALU.mult, op1=ALU.mult,
                    )
                    nxt = const_pool.tile([C, HALF], fp32, tag=f"nt{h}{it}r")
                    nc.vector.scalar_tensor_tensor(
                        out=nxt[:], in0=v[:], scalar=1.5, in1=cur,
                        op0=ALU.add, op1=ALU.mult,
                    )
                    cur = nxt[:]
                for j in range(h * HALF, (h + 1) * HALF):
                    r = cur[:, j - h * HALF : j - h * HALF + 1]
                    # Half 0 runs fully on DVE so its ACT ops never sit in
                    # front of later tanhs in the ACT FIFO (which would
                    # stall pair 3's whole pipeline).  Half 1 is the kernel
                    # tail, where ACT is done: split it ACT/DVE.
                    if h == 1 and j >= NE - 2:
                        nc.scalar.mul(out_sb[:, j, :], psy_tiles[j], r)
                    else:
                        nc.vector.tensor_scalar_mul(
                            out_sb[:, j, :], psy_tiles[j], r
                        )
                nc.sync.dma_start(y[:, sl, :], out_sb[:, sl, :])

            pair_body(0)
            pair_body(1)
            norm_half(0)
            pair_body(2)
            pair_body(3)
            norm_half(1)

    nc.compile()
    return nc


def _get_nc():
    if "nc" not in _compiled:
        _compiled["nc"] = _build_nc()
    return _compiled["nc"]


def _route(relation_ids):
    """Host-side routing: stable-sort samples by relation; per-expert
    sample positions, each group must fit the C=128 capacity."""
    order = np.argsort(relation_ids, kind="stable")
    counts = np.bincount(relation_ids, minlength=E)
    if counts.max() > C:
        raise ValueError(
            f"expert count {counts.max()} exceeds capacity {C}; "
            "kernel was compiled for capacity 128"
        )
    starts = np.zeros(E + 1, dtype=np.int64)
    np.cumsum(counts, out=starts[1:])
    return [order[starts[e] : starts[e + 1]] for e in range(E)]


def kernel(entity_ids, relation_ids, emb_table, W1, b1, W2, b2):
    from concourse.bass_utils import run_bass_kernel_spmd

    entity_ids = np.asarray(entity_ids).astype(np.int64)
    relation_ids = np.asarray(relation_ids).astype(np.int64)
    emb_table = np.asarray(emb_table, dtype=np.float32)
    W1 = np.asarray(W1, dtype=np.float32)
    b1 = np.asarray(b1, dtype=np.float32)
    W2 = np.asarray(W2, dtype=np.float32)
    b2 = np.asarray(b2, dtype=np.float32)

    per_expert_pos = _route(relation_ids)

    in_maps = []
    for c in range(N_CORES):
        lo, hi = c * NE, (c + 1) * NE
        # host gather + transpose: X^T chunks, capacity-padded, bf16
        xt_host = np.zeros((128, NE, 2, C), dtype=BF16)
        for j, e in enumerate(range(lo, hi)):
            pos = per_expert_pos[e]
            if len(pos):
                xt = emb_table[entity_ids[pos]].T.astype(BF16)  # [D, n]
                xt_host[:, j, 0, : len(pos)] = xt[0:128]
                xt_host[:, j, 1, : len(pos)] = xt[128:256]

        w1h = W1[lo:hi].reshape(NE, 2, 128, HD).transpose(0, 2, 1, 3)
        w2h = W2[lo:hi].reshape(NE, 2, 128, D).transpose(0, 2, 1, 3)
        w12_host = np.ascontiguousarray(
            np.concatenate([w1h, w2h], axis=2).transpose(1, 0, 2, 3)
        ).astype(BF16)                                  # [128, NE, 4, H]
        bias_host = np.ascontiguousarray(
            np.concatenate(
                [b1[lo:hi].reshape(NE, 2 * 128), b2[lo:hi]], axis=1
            ).reshape(1, NE, 2 * 128 + D)
        ).astype(BF16)
        in_maps.append(
            {
                "xt": np.ascontiguousarray(xt_host),
                "w12": w12_host,
                "bias": bias_host,
            }
        )

    nc = _get_nc()
    res = run_bass_kernel_spmd(nc, in_maps, core_ids=list(range(N_CORES)))
    _compiled["last_results"] = res

    out = np.empty((B, D), dtype=np.float32)
    for c in range(N_CORES):
        yc = np.asarray(res.results[c]["y"])           # [C, NE, D] bf16
        for j in range(NE):
            pos = per_expert_pos[c * NE + j]
            out[pos] = yc[: len(pos), j, :].astype(np.float32)
    return out


# revision 33
# speedup vs baseline: 1.0935x; 1.0935x over previous
"""Trainium2 Bass kernel for MoE-routed embedding MLP (nn_KML_24300924961295).

Model (B=4096, E=64 experts, D=H=256, vocab 100000):
    x = emb_table[entity_ids]                    # [B, D]
    h = tanh(x @ W1[rel] + b1[rel])              # [B, H]
    y = h @ W2[rel] + b2[rel]                    # [B, D]
    out = y / ||y||_2 (row-wise)

Sharding: experts are sharded across the 8 cores (core c owns experts
8c..8c+7); samples are routed on the host to the core owning their
relation, each expert group padded to a fixed capacity of C=128 samples
so all cores run one identical SPMD program.  The embedding rows are
gathered AND transposed on the host (X^T per expert), so the device
sees dense bf16 operands and does no indirect DMA and no PE transposes.

Per-core device pipeline (all matmul operands bf16, PSUM fp32), for
each pair of experts (2j, 2j+1):
    H^T [h,c] <- matmul(lhsT=W1 chunk, rhs=X^T chunk) accum over d,
                 + rank-1 bias matmul (b1 row x ones)      -> ps_h2
    ht        <- one ACT Tanh over the whole [128, 512] pair tile
    Y   [c,d] <- matmul(lhsT=H^T chunk, rhs=W2 rows) + rank-1 (ones x b2)
    s2  [c,1] <- DVE tensor_tensor_reduce(psy * psy)  (row sum of squares)
Then per half (4 experts): rsqrt on DVE only (0x5f3759df magic seed +
2 Newton steps), per-expert scale on ACT (Copy w/ per-partition scale,
fp32 PSUM -> bf16 SBUF), one 256 KiB output DMA.  Host upcasts to fp32.
"""

import numpy as np
from contextlib import ExitStack

import ml_dtypes

# ---- problem constants (hardcoded per the task contract) ----
B = 4096
E = 64
D = 256
HD = 256
N_CORES = 8
NE = E // N_CORES          # experts per core
C = 128                    # capacity (samples) per expert
HALF = NE // 2

BF16 = ml_dtypes.bfloat16
RSQRT_MAGIC = 0x5F3759DF

_compiled = {}


def _build_nc(C=C):
    """Build + schedule the single-core SPMD Bass program for capacity C
    (a multiple of 32, <=128; the parameter shadows the default above)."""
    import concourse.bass as bass  # noqa: F401  (kept for parity with docs)
    import concourse.bacc as bacc
    import concourse.tile as tile
    from concourse import mybir

    fp32 = mybir.dt.float32
    bf16 = mybir.dt.bfloat16
    u32 = mybir.dt.uint32
    AF = mybir.ActivationFunctionType
    ALU = mybir.AluOpType

    nc = bacc.Bacc("TRN2", target_bir_lowering=False, debug=False)

    # X^T: [d-in-chunk(128 part), expert, d-chunk, sample]
    xt_in = nc.dram_tensor("xt", [128, NE, 2, C], bf16, kind="ExternalInput").ap()
    # w12[e, p, 0:2, :] = W1 K-chunks, w12[e, p, 2:4, :] = W2 H-chunks
    w12 = nc.dram_tensor("w12", [NE, 128, 4, HD], bf16, kind="ExternalInput").ap()
    # b1 rows for the rank-1 bias matmul: [1, expert, h-chunk, 128]
    b1 = nc.dram_tensor("b1", [1, NE, 2, 128], bf16, kind="ExternalInput").ap()
    b2 = nc.dram_tensor("b2", [1, NE, D], bf16, kind="ExternalInput").ap()
    # output row-major per sample slot: [sample, expert, D]
    y = nc.dram_tensor("y", [C, NE, D], bf16, kind="ExternalOutput").ap()

    with tile.TileContext(nc) as tc:
        with ExitStack() as ctx:
            const_pool = ctx.enter_context(tc.tile_pool(name="const", bufs=1))
            w_pool = ctx.enter_context(tc.tile_pool(name="wp", bufs=NE))
            ht_pool = ctx.enter_context(tc.tile_pool(name="htp", bufs=3))
            psh_pool = ctx.enter_context(
                tc.tile_pool(name="psh", bufs=2, space="PSUM")
            )
            psy_pool = ctx.enter_context(
                tc.tile_pool(name="psy", bufs=1, space="PSUM")
            )
            sq_pool = ctx.enter_context(tc.tile_pool(name="sqp", bufs=2))

            # scalar (ACT) HWDGE ring: small consts + the second xt half.
            b1_sb = const_pool.tile([1, NE, 2, 128], bf16)
            nc.scalar.dma_start(b1_sb[:], b1[:])
            b2_sb = const_pool.tile([1, NE, D], bf16)
            nc.scalar.dma_start(b2_sb[:], b2[:])
            xt_all = const_pool.tile([128, NE, 2, C], bf16)
            nc.scalar.dma_start(xt_all[:, HALF:], xt_in[:, HALF:])

            # sync (SP) HWDGE ring: first xt half, then per-expert weights.
            nc.sync.dma_start(xt_all[:, 0:HALF], xt_in[:, 0:HALF])
            w_tiles = []
            for j in range(NE):
                wt = w_pool.tile([128, 4, HD], bf16)
                nc.sync.dma_start(wt[:], w12[j])
                w_tiles.append(wt)

            ones1 = const_pool.tile([1, C], bf16)
            nc.gpsimd.memset(ones1[:], 1.0)
            kmag = const_pool.tile([C, HALF], u32)
            nc.gpsimd.memset(kmag[:], RSQRT_MAGIC)

            s2_all = const_pool.tile([C, NE], fp32)
            out_sb = const_pool.tile([C, NE, D], bf16)

            psy_tiles = []

            def pair_body(t):
                """Experts 2t, 2t+1: H^T + tanh + Y + row sum-of-squares."""
                ps_h2 = psh_pool.tile([128, 2, 2, C], fp32, tag="psh2")
                for j2 in range(2):
                    j = 2 * t + j2
                    wt = w_tiles[j]
                    for hc in range(2):
                        for dc in range(2):
                            nc.tensor.matmul(
                                ps_h2[:, j2, hc, :],
                                lhsT=wt[:, dc, hc * 128 : (hc + 1) * 128],
                                rhs=xt_all[:, j, dc, :],
                                start=(dc == 0),
                                stop=False,
                            )
                        nc.tensor.matmul(
                            ps_h2[:, j2, hc, :],
                            lhsT=b1_sb[:, j, hc, :],
                            rhs=ones1[:],
                            start=False,
                            stop=True,
                        )
                ht2 = ht_pool.tile([128, 2, 2, C], bf16)
                nc.scalar.activation(ht2[:], ps_h2[:], AF.Tanh)
                ps_y2 = psy_pool.tile([C, 2, D], fp32, tag=f"psy{t}")
                for j2 in range(2):
                    j = 2 * t + j2
                    wt = w_tiles[j]
                    ps_y = ps_y2[:, j2, :]
                    nc.tensor.matmul(
                        ps_y, lhsT=ht2[:, j2, 0, :], rhs=wt[:, 2, :],
                        start=True, stop=False,
                    )
                    nc.tensor.matmul(
                        ps_y, lhsT=ht2[:, j2, 1, :], rhs=wt[:, 3, :],
                        start=False, stop=False,
                    )
                    nc.tensor.matmul(
                        ps_y, lhsT=ones1[:], rhs=b2_sb[:, j, :],
                        start=False, stop=True,
                    )
                    psy_tiles.append(ps_y)
                    if j in (HALF - 1, NE - 1):
                        # last expert of each half: square on DVE so the ACT
                        # queue isn't the tail; DVE may read PSUM only once
                        # per instruction, so copy out, square, reduce
                        ysb = sq_pool.tile([C, D], bf16, tag="ysb")
                        nc.vector.tensor_copy(ysb[:], ps_y)
                        ysq = sq_pool.tile([C, D], fp32, tag="ysq")
                        nc.vector.tensor_mul(ysq[:], ysb[:], ysb[:])
                        nc.vector.tensor_reduce(
                            out=s2_all[:, j : j + 1], in_=ysq[:],
                            axis=mybir.AxisListType.X, op=ALU.add,
                        )
                    else:
                        # ACT Square (+row accumulate); square is in the same
                        # table set as tanh, so no ACT table switch
                        sq = sq_pool.tile([C, D], bf16, tag="sqa")
                        nc.scalar.activation(
                            sq[:], ps_y, AF.Square,
                            accum_out=s2_all[:, j : j + 1],
                        )

            def norm_half(h):
                """rsqrt of s2 (DVE-only), ACT-scale the 4 experts, store."""
                sl = slice(h * HALF, (h + 1) * HALF)
                s2u = s2_all[:, sl].bitcast(u32)
                sh = const_pool.tile([C, HALF], u32, tag=f"sh{h}")
                nc.vector.tensor_scalar(
                    out=sh[:], in0=s2u, scalar1=1, scalar2=None,
                    op0=ALU.logical_shift_right,
                )
                sd = const_pool.tile([C, HALF], u32, tag=f"sd{h}")
                nc.vector.tensor_tensor(
                    out=sd[:], in0=kmag[:], in1=sh[:], op=ALU.subtract
                )
                cur = sd[:].bitcast(fp32)
                s2 = s2_all[:, sl]
                # Newton: r' = r*(1.5 - 0.5*s2*r^2), 3 DVE ops per step
                for it in range(2):
                    u = const_pool.tile([C, HALF], fp32, tag=f"nt{h}{it}u")
                    nc.vector.tensor_mul(u[:], cur, s2)
                    v = const_pool.tile([C, HALF], fp32, tag=f"nt{h}{it}v")
                    nc.vector.scalar_tensor_tensor(
                        out=v[:], in0=u[:], scalar=-0.5, in1=cur,
                        op0=ALU.mult, op1=ALU.mult,
                    )
                    nxt = const_pool.tile([C, HALF], fp32, tag=f"nt{h}{it}r")
                    nc.vector.scalar_tensor_tensor(
                        out=nxt[:], in0=v[:], scalar=1.5, in1=cur,
                        op0=ALU.add, op1=ALU.mult,
                    )
                    cur = nxt[:]
                for j in range(h * HALF, (h + 1) * HALF):
                    r = cur[:, j - h * HALF : j - h * HALF + 1]
                    if h == 1 and j >= NE - 2:
                        # tail half: split scales across ACT + DVE so the
                        # final norm chain isn't serial on one engine
                        nc.scalar.mul(out_sb[:, j, :], psy_tiles[j], r)
                    else:
                        nc.vector.tensor_scalar_mul(
                            out_sb[:, j, :], psy_tiles[j], r
                        )
                nc.sync.dma_start(y[:, sl, :], out_sb[:, sl, :])

            pair_body(0)
            pair_body(1)
            pair_body(2)
            norm_half(0)
            pair_body(3)
            norm_half(1)

    nc.compile()
    return nc


def _get_nc(cap):
    key = f"nc{cap}"
    if key not in _compiled:
        _compiled[key] = _build_nc(cap)
    return _compiled[key]


def _route(relation_ids):
    """Host-side routing: stable-sort samples by relation; per-expert
    sample positions plus the padded capacity (multiple of 32, <=128)."""
    order = np.argsort(relation_ids, kind="stable")
    counts = np.bincount(relation_ids, minlength=E)
    cap = int(-(-max(1, counts.max()) // 32) * 32)
    if cap > 128:
        raise ValueError(
            f"expert count {counts.max()} exceeds the 128-sample capacity"
        )
    starts = np.zeros(E + 1, dtype=np.int64)
    np.cumsum(counts, out=starts[1:])
    return [order[starts[e] : starts[e + 1]] for e in range(E)], cap


def kernel(entity_ids, relation_ids, emb_table, W1, b1, W2, b2):
    from concourse.bass_utils import run_bass_kernel_spmd

    entity_ids = np.asarray(entity_ids).astype(np.int64)
    relation_ids = np.asarray(relation_ids).astype(np.int64)
    emb_table = np.asarray(emb_table, dtype=np.float32)
    W1 = np.asarray(W1, dtype=np.float32)
    b1 = np.asarray(b1, dtype=np.float32)
    W2 = np.asarray(W2, dtype=np.float32)
    b2 = np.asarray(b2, dtype=np.float32)

    per_expert_pos, cap = _route(relation_ids)

    in_maps = []
    for c in range(N_CORES):
        lo, hi = c * NE, (c + 1) * NE
        # host gather + transpose: X^T chunks, capacity-padded, bf16
        xt_host = np.zeros((128, NE, 2, cap), dtype=BF16)
        for j, e in enumerate(range(lo, hi)):
            pos = per_expert_pos[e]
            if len(pos):
                xt = emb_table[entity_ids[pos]].T.astype(BF16)  # [D, n]
                xt_host[:, j, 0, : len(pos)] = xt[0:128]
                xt_host[:, j, 1, : len(pos)] = xt[128:256]

        w1h = W1[lo:hi].reshape(NE, 2, 128, HD).transpose(0, 2, 1, 3)
        w2h = W2[lo:hi].reshape(NE, 2, 128, D).transpose(0, 2, 1, 3)
        w12_host = np.ascontiguousarray(
            np.concatenate([w1h, w2h], axis=2)
        ).astype(BF16)                                  # [NE, 128, 4, H]
        b1_host = np.ascontiguousarray(
            b1[lo:hi].reshape(1, NE, 2, 128)
        ).astype(BF16)
        b2_host = np.ascontiguousarray(b2[lo:hi].reshape(1, NE, D)).astype(BF16)
        in_maps.append(
            {
                "xt": np.ascontiguousarray(xt_host),
                "w12": w12_host,
                "b1": b1_host,
                "b2": b2_host,
            }
        )

    nc = _get_nc(cap)
    res = run_bass_kernel_spmd(nc, in_maps, core_ids=list(range(N_CORES)))
    _compiled["last_results"] = res

    out = np.empty((B, D), dtype=np.float32)
    for c in range(N_CORES):
        yc = np.asarray(res.results[c]["y"])           # [C, NE, D] bf16
        for j in range(NE):
            pos = per_expert_pos[c * NE + j]
            out[pos] = yc[: len(pos), j, :].astype(np.float32)
    return out


# revision 34
# speedup vs baseline: 1.1412x; 1.0437x over previous
"""Trainium2 Bass kernel for MoE-routed embedding MLP (nn_KML_24300924961295).

Model (B=4096, E=64 experts, D=H=256, vocab 100000):
    x = emb_table[entity_ids]                    # [B, D]
    h = tanh(x @ W1[rel] + b1[rel])              # [B, H]
    y = h @ W2[rel] + b2[rel]                    # [B, D]
    out = y / ||y||_2 (row-wise)

Sharding: experts are sharded across the 8 cores (core c owns experts
8c..8c+7); samples are routed on the host to the core owning their
relation, each expert group padded to a fixed capacity of C=128 samples
so all cores run one identical SPMD program.  The embedding rows are
gathered AND transposed on the host (X^T per expert), so the device
sees dense bf16 operands and does no indirect DMA and no PE transposes.

Per-core device pipeline (all matmul operands bf16, PSUM fp32), for
each pair of experts (2j, 2j+1):
    H^T [h,c] <- matmul(lhsT=W1 chunk, rhs=X^T chunk) accum over d,
                 + rank-1 bias matmul (b1 row x ones)      -> ps_h2
    ht        <- one ACT Tanh over the whole [128, 512] pair tile
    Y   [c,d] <- matmul(lhsT=H^T chunk, rhs=W2 rows) + rank-1 (ones x b2)
    s2  [c,1] <- DVE tensor_tensor_reduce(psy * psy)  (row sum of squares)
Then per half (4 experts): rsqrt on DVE only (0x5f3759df magic seed +
2 Newton steps), per-expert scale on ACT (Copy w/ per-partition scale,
fp32 PSUM -> bf16 SBUF), one 256 KiB output DMA.  Host upcasts to fp32.
"""

import numpy as np
from contextlib import ExitStack

import ml_dtypes

# ---- problem constants (hardcoded per the task contract) ----
B = 4096
E = 64
D = 256
HD = 256
N_CORES = 8
NE = E // N_CORES          # experts per core
C = 128                    # capacity (samples) per expert
HALF = NE // 2

BF16 = ml_dtypes.bfloat16
RSQRT_MAGIC = 0x5F3759DF

_compiled = {}


def _build_nc(C=C):
    """Build + schedule the single-core SPMD Bass program for capacity C
    (a multiple of 32, <=128; the parameter shadows the default above)."""
    import concourse.bass as bass  # noqa: F401  (kept for parity with docs)
    import concourse.bacc as bacc
    import concourse.tile as tile
    from concourse import mybir

    fp32 = mybir.dt.float32
    bf16 = mybir.dt.bfloat16
    u32 = mybir.dt.uint32
    AF = mybir.ActivationFunctionType
    ALU = mybir.AluOpType

    nc = bacc.Bacc("TRN2", target_bir_lowering=False, debug=False)

    # X^T: [d-in-chunk(128 part), expert, d-chunk, sample]
    xt_in = nc.dram_tensor("xt", [128, NE, 2, C], bf16, kind="ExternalInput").ap()
    # w12[e, p, 0:2, :] = W1 K-chunks, w12[e, p, 2:4, :] = W2 H-chunks
    w12 = nc.dram_tensor("w12", [NE, 128, 4, HD], bf16, kind="ExternalInput").ap()
    # b1 rows for the rank-1 bias matmul: [1, expert, h-chunk, 128]
    b1 = nc.dram_tensor("b1", [1, NE, 2, 128], bf16, kind="ExternalInput").ap()
    b2 = nc.dram_tensor("b2", [1, NE, D], bf16, kind="ExternalInput").ap()
    # output row-major per sample slot: [sample, expert, D]
    y = nc.dram_tensor("y", [C, NE, D], bf16, kind="ExternalOutput").ap()

    with tile.TileContext(nc) as tc:
        with ExitStack() as ctx:
            const_pool = ctx.enter_context(tc.tile_pool(name="const", bufs=1))
            w_pool = ctx.enter_context(tc.tile_pool(name="wp", bufs=NE))
            ht_pool = ctx.enter_context(tc.tile_pool(name="htp", bufs=3))
            psh_pool = ctx.enter_context(
                tc.tile_pool(name="psh", bufs=2, space="PSUM")
            )
            psy_pool = ctx.enter_context(
                tc.tile_pool(name="psy", bufs=1, space="PSUM")
            )
            sq_pool = ctx.enter_context(tc.tile_pool(name="sqp", bufs=2))

            # scalar (ACT) HWDGE ring: small consts + the second xt half.
            b1_sb = const_pool.tile([1, NE, 2, 128], bf16)
            nc.scalar.dma_start(b1_sb[:], b1[:])
            b2_sb = const_pool.tile([1, NE, D], bf16)
            nc.scalar.dma_start(b2_sb[:], b2[:])
            xt_all = const_pool.tile([128, NE, 2, C], bf16)
            nc.scalar.dma_start(xt_all[:, HALF:], xt_in[:, HALF:])

            # sync (SP) HWDGE ring: first xt half, then per-expert weights.
            nc.sync.dma_start(xt_all[:, 0:HALF], xt_in[:, 0:HALF])
            w_tiles = []
            for j in range(NE):
                wt = w_pool.tile([128, 4, HD], bf16)
                nc.sync.dma_start(wt[:], w12[j])
                w_tiles.append(wt)

            ones1 = const_pool.tile([1, C], bf16)
            nc.gpsimd.memset(ones1[:], 1.0)
            kmag = const_pool.tile([C, HALF], u32)
            nc.gpsimd.memset(kmag[:], RSQRT_MAGIC)

            s2_all = const_pool.tile([C, NE], fp32)
            out_sb = const_pool.tile([C, NE, D], bf16)

            psy_tiles = []

            def pair_body(t):
                """Experts 2t, 2t+1: H^T + tanh + Y + row sum-of-squares."""
                ps_h2 = psh_pool.tile([128, 2, 2, C], fp32, tag="psh2")
                for j2 in range(2):
                    j = 2 * t + j2
                    wt = w_tiles[j]
                    for hc in range(2):
                        for dc in range(2):
                            nc.tensor.matmul(
                                ps_h2[:, j2, hc, :],
                                lhsT=wt[:, dc, hc * 128 : (hc + 1) * 128],
                                rhs=xt_all[:, j, dc, :],
                                start=(dc == 0),
                                stop=False,
                            )
                        nc.tensor.matmul(
                            ps_h2[:, j2, hc, :],
                            lhsT=b1_sb[:, j, hc, :],
                            rhs=ones1[:],
                            start=False,
                            stop=True,
                        )
                ht2 = ht_pool.tile([128, 2, 2, C], bf16)
                nc.scalar.activation(ht2[:], ps_h2[:], AF.Tanh)
                ps_y2 = psy_pool.tile([C, 2, D], fp32, tag=f"psy{t}")
                for j2 in range(2):
                    j = 2 * t + j2
                    wt = w_tiles[j]
                    ps_y = ps_y2[:, j2, :]
                    nc.tensor.matmul(
                        ps_y, lhsT=ht2[:, j2, 0, :], rhs=wt[:, 2, :],
                        start=True, stop=False,
                    )
                    nc.tensor.matmul(
                        ps_y, lhsT=ht2[:, j2, 1, :], rhs=wt[:, 3, :],
                        start=False, stop=False,
                    )
                    nc.tensor.matmul(
                        ps_y, lhsT=ones1[:], rhs=b2_sb[:, j, :],
                        start=False, stop=True,
                    )
                    psy_tiles.append(ps_y)
                    # ACT Square (+row accumulate); square is in the same
                    # table set as tanh, so no ACT table switch, and ACT
                    # has slack under the PE phase
                    sq = sq_pool.tile([C, D], bf16, tag="sqa")
                    nc.scalar.activation(
                        sq[:], ps_y, AF.Square,
                        accum_out=s2_all[:, j : j + 1],
                    )

            def norm_half(h):
                """rsqrt of s2 (DVE-only), ACT-scale the 4 experts, store."""
                sl = slice(h * HALF, (h + 1) * HALF)
                s2u = s2_all[:, sl].bitcast(u32)
                sh = const_pool.tile([C, HALF], u32, tag=f"sh{h}")
                nc.vector.tensor_scalar(
                    out=sh[:], in0=s2u, scalar1=1, scalar2=None,
                    op0=ALU.logical_shift_right,
                )
                sd = const_pool.tile([C, HALF], u32, tag=f"sd{h}")
                nc.vector.tensor_tensor(
                    out=sd[:], in0=kmag[:], in1=sh[:], op=ALU.subtract
                )
                cur = sd[:].bitcast(fp32)
                s2 = s2_all[:, sl]
                # one Newton step: r' = r*(1.5 - 0.5*s2*r^2) -> ~0.2% max
                # rel err on the row norm, well inside the error budget
                for it in range(1):
                    u = const_pool.tile([C, HALF], fp32, tag=f"nt{h}{it}u")
                    nc.vector.tensor_mul(u[:], cur, s2)
                    v = const_pool.tile([C, HALF], fp32, tag=f"nt{h}{it}v")
                    nc.vector.scalar_tensor_tensor(
                        out=v[:], in0=u[:], scalar=-0.5, in1=cur,
                        op0=ALU.mult, op1=ALU.mult,
                    )
                    nxt = const_pool.tile([C, HALF], fp32, tag=f"nt{h}{it}r")
                    nc.vector.scalar_tensor_tensor(
                        out=nxt[:], in0=v[:], scalar=1.5, in1=cur,
                        op0=ALU.add, op1=ALU.mult,
                    )
                    cur = nxt[:]
                for j in range(h * HALF, (h + 1) * HALF):
                    r = cur[:, j - h * HALF : j - h * HALF + 1]
                    if h == 1 and j >= NE - 2:
                        # tail half: split scales across ACT + DVE so the
                        # final norm chain isn't serial on one engine
                        nc.scalar.mul(out_sb[:, j, :], psy_tiles[j], r)
                    else:
                        nc.vector.tensor_scalar_mul(
                            out_sb[:, j, :], psy_tiles[j], r
                        )
                nc.sync.dma_start(y[:, sl, :], out_sb[:, sl, :])

            pair_body(0)
            pair_body(1)
            norm_half(0)
            pair_body(2)
            pair_body(3)
            norm_half(1)

    nc.compile()
    return nc


def _get_nc(cap):
    key = f"nc{cap}"
    if key not in _compiled:
        _compiled[key] = _build_nc(cap)
    return _compiled[key]


def _route(relation_ids):
    """Host-side routing: stable-sort samples by relation; per-expert
    sample positions plus the padded capacity (multiple of 32, <=128)."""
    order = np.argsort(relation_ids, kind="stable")
    counts = np.bincount(relation_ids, minlength=E)
    cap = int(-(-max(1, counts.max()) // 32) * 32)
    if cap > 128:
        raise ValueError(
            f"expert count {counts.max()} exceeds the 128-sample capacity"
        )
    starts = np.zeros(E + 1, dtype=np.int64)
    np.cumsum(counts, out=starts[1:])
    return [order[starts[e] : starts[e + 1]] for e in range(E)], cap


def kernel(entity_ids, relation_ids, emb_table, W1, b1, W2, b2):
    from concourse.bass_utils import run_bass_kernel_spmd

    entity_ids = np.asarray(entity_ids).astype(np.int64)
    relation_ids = np.asarray(relation_ids).astype(np.int64)
    emb_table = np.asarray(emb_table, dtype=np.float32)
    W1 = np.asarray(W1, dtype=np.float32)
    b1 = np.asarray(b1, dtype=np.float32)
    W2 = np.asarray(W2, dtype=np.float32)
    b2 = np.asarray(b2, dtype=np.float32)

    per_expert_pos, cap = _route(relation_ids)

    in_maps = []
    for c in range(N_CORES):
        lo, hi = c * NE, (c + 1) * NE
        # host gather + transpose: X^T chunks, capacity-padded, bf16
        xt_host = np.zeros((128, NE, 2, cap), dtype=BF16)
        for j, e in enumerate(range(lo, hi)):
            pos = per_expert_pos[e]
            if len(pos):
                xt = emb_table[entity_ids[pos]].T.astype(BF16)  # [D, n]
                xt_host[:, j, 0, : len(pos)] = xt[0:128]
                xt_host[:, j, 1, : len(pos)] = xt[128:256]

        w1h = W1[lo:hi].reshape(NE, 2, 128, HD).transpose(0, 2, 1, 3)
        w2h = W2[lo:hi].reshape(NE, 2, 128, D).transpose(0, 2, 1, 3)
        w12_host = np.ascontiguousarray(
            np.concatenate([w1h, w2h], axis=2)
        ).astype(BF16)                                  # [NE, 128, 4, H]
        b1_host = np.ascontiguousarray(
            b1[lo:hi].reshape(1, NE, 2, 128)
        ).astype(BF16)
        b2_host = np.ascontiguousarray(b2[lo:hi].reshape(1, NE, D)).astype(BF16)
        in_maps.append(
            {
                "xt": np.ascontiguousarray(xt_host),
                "w12": w12_host,
                "b1": b1_host,
                "b2": b2_host,
            }
        )

    nc = _get_nc(cap)
    res = run_bass_kernel_spmd(nc, in_maps, core_ids=list(range(N_CORES)))
    _compiled["last_results"] = res

    out = np.empty((B, D), dtype=np.float32)
    for c in range(N_CORES):
        yc = np.asarray(res.results[c]["y"])           # [C, NE, D] bf16
        for j in range(NE):
            pos = per_expert_pos[c * NE + j]
            out[pos] = yc[: len(pos), j, :].astype(np.float32)
    return out
